# revision 1
# baseline (speedup 1.0000x reference)
"""TRN2 Bass kernel for nn_MultiHeadAttention (B=2, S=2048, D=1024, H=16, causal).

Sharding: 8 cores = (batch b in {0,1}) x (head-group hg in {0..3}, 4 heads each).
Each core computes Q/K/V projections for its head group (full S), causal
attention for its 4 heads, and a partial output projection against its
256-column slice of w_o.  Host sums the 4 partial Y per batch and adds b_o.

Device dataflow (per core):
  qT,kT (fp32r, host-pretransposed [D,S]) -> Q^T,K^T [256,S] fp32r via PE;
  vT (fp16) -> V [S,256] fp16 via PE (+ bias via a K=1 ones-matmul).
  Per (q-tile of 128, head-pair): QK matmuls (K=64, the two heads ride
  disjoint PE row groups so they run concurrently), causal mask add on the
  diagonal block (DVE), exact per-tile row-max -> combined negated max (DVE),
  exp with bias=-max and fused row-sum via accum_out (ACT), 1/l (DVE) stashed
  as an extra E column, E^T via one batched fp16 SBUF->SBUF xbar DMA
  transpose, PV matmuls col-packed for the head pair into an O^T psum
  [128,128] (deferred one iteration so they queue behind the next QK), 1/l
  row broadcast via K=1 PE matmuls, O^T = pv * R (DVE) -> fp16 O^T.
  Tail: output projection Y[128,512] psum tiles (fp16 matmuls), copyback,
  DMA out as fp16 partials; host sums partials over head groups + b_o.
"""
import numpy as np

B, S, D, H = 2, 2048, 1024, 16
DK = D // H          # 64
HG = 4               # heads per core
DHG = HG * DK        # 256 head dims per core
P = 128
NQT = S // P         # 16 q tiles
KT = 512             # score tile width
MAXC = S // KT       # 4 score tiles per row max
SCALE = float(np.sqrt(DK))  # reference multiplies by sqrt(dk)
MSHIFT = 0.0         # extra shift on row-max (0: exact row max)
NEG = -1.0e30

_FP16 = "float16"
QK_F16 = False


def _build(reps=1, n_cores=8, stage=9, sc_bufs=6, pv_bufs=2, yp_bufs=4, ew_bufs=4, qk_f16=None):
    import concourse.bass as bass
    import concourse.mybir as mybir
    import concourse.tile as tile
    from concourse import bacc

    if qk_f16 is None:
        qk_f16 = QK_F16
    f32 = mybir.dt.float32
    f32r = mybir.dt.float32r
    f16 = getattr(mybir.dt, _FP16)
    fqk = f16 if qk_f16 else f32r

    nc = bacc.Bacc("TRN2", target_bir_lowering=False, debug=False,
                   num_devices=n_cores)

    qT = nc.dram_tensor("qT", [D, S], fqk, kind="ExternalInput")
    kT = nc.dram_tensor("kT", [D, S], fqk, kind="ExternalInput")
    vT = nc.dram_tensor("vT", [D, S], f16, kind="ExternalInput")
    wqT = nc.dram_tensor("wqT", [D, DHG], fqk, kind="ExternalInput")
    wkT = nc.dram_tensor("wkT", [D, DHG], fqk, kind="ExternalInput")
    wvT = nc.dram_tensor("wvT", [D, DHG], f16, kind="ExternalInput")
    woT = nc.dram_tensor("woT", [DHG, D], f16, kind="ExternalInput")
    bqc = nc.dram_tensor("bqc", [P, 2], f32, kind="ExternalInput")
    bkc = nc.dram_tensor("bkc", [P, 2], f32, kind="ExternalInput")
    bvr = nc.dram_tensor("bvr", [1, DHG], f16, kind="ExternalInput")
    mask = nc.dram_tensor("mask", [P, P], f32, kind="ExternalInput")
    y = nc.dram_tensor("y", [S, D], f16, kind="ExternalOutput")

    with tile.TileContext(nc) as tc:
        with (
            tc.tile_pool(name="wpool", bufs=1) as wpool,
            tc.tile_pool(name="persist", bufs=1) as persist,
            tc.tile_pool(name="io", bufs=4) as io,
            tc.tile_pool(name="ework", bufs=ew_bufs) as ework,
            tc.tile_pool(name="stat", bufs=12) as stat,
            tc.tile_pool(name="ysb", bufs=2) as ysb,
        ):
            loop_ctx = tc.For_i(0, reps, 1) if reps != 1 else None
            if loop_ctx is not None:
                loop_ctx.__enter__()

            # ---- static weights / consts ----
            wq_sb = wpool.tile([P, D // P, DHG], fqk, tag="wq")
            nc.sync.dma_start(wq_sb[:], wqT.rearrange("(c p) n -> p c n", p=P))
            wk_sb = wpool.tile([P, D // P, DHG], fqk, tag="wk")
            nc.sync.dma_start(wk_sb[:], wkT.rearrange("(c p) n -> p c n", p=P))
            wv_sb = wpool.tile([P, D // P, DHG], f16, tag="wv")
            nc.sync.dma_start(wv_sb[:], wvT.rearrange("(c p) n -> p c n", p=P))
            wo_sb = wpool.tile([P, DHG // P, D], f16, tag="wo")
            nc.sync.dma_start(wo_sb[:], woT.rearrange("(c p) n -> p c n", p=P))
            bq_sb = wpool.tile([P, 2], f32, tag="bq")
            nc.sync.dma_start(bq_sb[:], bqc[:])
            bk_sb = wpool.tile([P, 2], f32, tag="bk")
            nc.sync.dma_start(bk_sb[:], bkc[:])
            bv_sb = wpool.tile([1, DHG], f16, tag="bv")
            nc.sync.dma_start(bv_sb[:], bvr[:])
            mask_sb = wpool.tile([P, P], f32, tag="mask")
            nc.sync.dma_start(mask_sb[:], mask[:])
            ones_sb = wpool.tile([1, P], f16, tag="ones")
            nc.vector.memset(ones_sb[:], 1.0)

            # ---- persistent activations ----
            # Q^T / K^T: chunk m holds head-dims [128m, 128m+128) over full S
            QTs = [persist.tile([P, S], fqk, tag=f"QT{m}", name=f"QT{m}") for m in range(2)]
            KTs = [persist.tile([P, S], fqk, tag=f"KT{m}", name=f"KT{m}") for m in range(2)]
            Vsb = persist.tile([P, NQT, DHG], f16, tag="V")       # V[128k, chunk, 256]
            OTs = [persist.tile([P, S], f16, tag=f"OT{m}", name=f"OT{m}") for m in range(2)]

            # ================= projections =================
            with tc.tile_pool(name="pj", bufs=2, space="PSUM") as pj:
                # Q^T and K^T: out chunk m, s-tile of 512
                for (dst, wsb, bsb, src) in ((QTs, wq_sb, bq_sb, qT),
                                             (KTs, wk_sb, bk_sb, kT)):
                    for st in range(S // KT):
                        xt = io.tile([P, D // P, KT], fqk, tag="xstream")
                        nc.sync.dma_start(
                            xt[:],
                            src.rearrange("(c p) s -> p c s", p=P)[
                                :, :, st * KT:(st + 1) * KT],
                        )
                        for m in range(2):
                            ps = pj.tile([P, KT], f32, tag="pjp")
                            for c in range(D // P):
                                nc.tensor.matmul(
                                    ps[:],
                                    wsb[:, c, m * P:(m + 1) * P],
                                    xt[:, c, :],
                                    start=(c == 0), stop=(c == D // P - 1),
                                )
                            nc.scalar.activation(
                                dst[m][:, st * KT:(st + 1) * KT], ps[:],
                                mybir.ActivationFunctionType.Identity,
                                bias=bsb[:, m:m + 1], scale=1.0,
                            )
                # V: natural [S,256] by k-chunk
                for st in range(NQT):
                    xt = io.tile([P, D // P, P], f16, tag="vstream")
                    nc.sync.dma_start(
                        xt[:],
                        vT.rearrange("(c p) s -> p c s", p=P)[
                            :, :, st * P:(st + 1) * P],
                    )
                    ps = pj.tile([P, DHG], f32, tag="pjv")
                    for c in range(D // P):
                        nc.tensor.matmul(
                            ps[:], xt[:, c, :], wv_sb[:, c, :],
                            start=(c == 0), stop=False,
                        )
                    nc.tensor.matmul(
                        ps[:], ones_sb[:], bv_sb[:],
                        start=False, stop=True,
                    )
                    nc.any.tensor_copy(Vsb[:, st, :], ps[:])

            # ================= attention + out-projection =================
            with (
                tc.tile_pool(name="sc", bufs=sc_bufs, space="PSUM") as scp,
                tc.tile_pool(name="pv", bufs=pv_bufs, space="PSUM") as pvp,
            ):
                def emit_pv_pe(i_, pr_, ETab_):
                    # PV pair: col-packed, heads A/B on col groups 0/1;
                    # emitted one iteration late so these PE ops queue
                    # behind the NEXT iteration's QK (hides the E^T DMA)
                    pv = pvp.tile([P, P], f32, tag="pv", name="pv")
                    for c in range(i_ + 1):
                        for ab in range(2):
                            nc.tensor.matmul(
                                pv[ab * DK:(ab + 1) * DK, :],
                                Vsb[:, c, (2 * pr_ + ab) * DK:
                                    (2 * pr_ + ab + 1) * DK],
                                ETab_[ab][:, c, :],
                                start=(c == 0), stop=(c == i_),
                                tile_position=(0, ab * DK),
                                skip_group_check=True,
                            )
                    # broadcast 1/l rows into R (PE K=1 matmuls)
                    rb = pvp.tile([P, P], f32, tag="pv", name="rb")
                    for ab in range(2):
                        nc.tensor.matmul(
                            rb[ab * DK:(ab + 1) * DK, :],
                            ones_sb[:, :DK],
                            ETab_[ab][0:1, i_ + 1, :],
                            start=True, stop=True,
                            tile_position=(0, ab * DK),
                            skip_group_check=True,
                        )
                    return (i_, pr_, pv, rb)

                def emit_pv_dve(i_, pr_, pv, rb):
                    rsb = ework.tile([P, P], f16, tag="rsb")
                    nc.any.tensor_copy(rsb[:], rb[:])
                    nc.vector.tensor_tensor(
                        OTs[pr_][:, i_ * P:(i_ + 1) * P], pv[:], rsb[:],
                        mybir.AluOpType.mult,
                    )

                pending = None
                for i in range(NQT):
                    w = (i + 1) * P          # causal row width
                    nt = (w + KT - 1) // KT  # score tiles
                    for pr in range(HG // 2):
                        # heads 2*pr, 2*pr+1 share Q^T/K^T chunk `pr` at
                        # partition offsets 0 / 64 -> their K=64 QK matmuls
                        # run on disjoint PE row groups (concurrent)

                        sc_ab = [[None] * nt, [None] * nt]

                        def qk_mm(ab, t):
                            tw = min(KT, w - t * KT)
                            po = ab * DK
                            sct = scp.tile([P, KT], f32, tag="sct",
                                           name=f"sct{ab}")
                            nc.tensor.matmul(
                                sct[:, :tw],
                                QTs[pr][po:po + DK, i * P:(i + 1) * P],
                                KTs[pr][po:po + DK, t * KT:t * KT + tw],
                                start=True, stop=True,
                            )
                            sc_ab[ab][t] = (sct, tw)

                        order = [nt - 1] + list(range(nt - 1))
                        if 2 * nt <= sc_bufs:
                            # A/B interleaved: disjoint row-groups overlap on PE
                            for t in order:
                                qk_mm(0, t)
                                qk_mm(1, t)
                        else:
                            for ab in range(2):
                                for t in order:
                                    qk_mm(ab, t)
                        pe_part = None
                        if stage >= 7 and pending is not None:
                            pe_part = emit_pv_pe(*pending)
                            pending = None
                        if stage < 2:
                            continue
                        ETab = []
                        for ab in range(2):
                            sc_tiles = sc_ab[ab]
                            # causal mask on the diagonal 128 block
                            sct, tw = sc_tiles[-1]
                            nc.vector.tensor_tensor(
                                sct[:, tw - P:tw], sct[:, tw - P:tw],
                                mask_sb[:], mybir.AluOpType.add,
                            )
                            # exact row-max (negated); single-tile rows skip
                            # the combine hop
                            negm = stat.tile([P, 1], f32, tag="negm")
                            if nt == 1:
                                sct, tw = sc_tiles[0]
                                nc.vector.tensor_reduce(
                                    negm[:], sct[:, :tw],
                                    axis=mybir.AxisListType.X,
                                    op=mybir.AluOpType.max, negate=True,
                                )
                            else:
                                msl = stat.tile([P, MAXC], f32, tag="msl")
                                for t, (sct, tw) in enumerate(sc_tiles):
                                    nc.vector.tensor_reduce(
                                        msl[:, t:t + 1], sct[:, :tw],
                                        axis=mybir.AxisListType.X,
                                        op=mybir.AluOpType.max,
                                    )
                                nc.vector.tensor_reduce(
                                    negm[:], msl[:, :nt],
                                    axis=mybir.AxisListType.X,
                                    op=mybir.AluOpType.max, negate=True,
                                )
                            if stage < 3:
                                continue
                            if MSHIFT:
                                nc.vector.tensor_scalar_add(
                                    negm[:], negm[:], -MSHIFT)
                            if stage < 4:
                                continue
                            # exp + row-sum
                            E = ework.tile([P, S + P], f16, tag="E")
                            lsl = stat.tile([P, MAXC], f32, tag="lsl")
                            for t, (sct, tw) in enumerate(sc_tiles):
                                nc.scalar.activation(
                                    E[:, t * KT:t * KT + tw], sct[:, :tw],
                                    mybir.ActivationFunctionType.Exp,
                                    bias=negm[:], scale=1.0,
                                    accum_out=lsl[:, t:t + 1],
                                )
                            if stage < 5:
                                continue
                            rl = stat.tile([P, 1], f32, tag="rl")
                            if nt == 1:
                                nc.vector.reciprocal(rl[:], lsl[:, 0:1])
                            else:
                                l = stat.tile([P, 1], f32, tag="l")
                                nc.vector.tensor_reduce(
                                    l[:], lsl[:, :nt],
                                    axis=mybir.AxisListType.X,
                                    op=mybir.AluOpType.add,
                                )
                                nc.vector.reciprocal(rl[:], l[:])
                            # stash 1/l as an extra E column; the batched
                            # transpose turns it into a [1,128] row for the
                            # PE broadcast below
                            nc.vector.tensor_copy(E[:, w:w + 1], rl[:])
                            if stage < 6:
                                continue
                            # transpose E (+rl column) -> E^T chunks
                            ET = ework.tile([P, NQT + 1, P], f16, tag="ET")
                            nc.sync.dma_start_transpose(
                                ET[:, :i + 2, :], E[:, :(i + 2) * P])
                            ETab.append(ET)
                        if pe_part is not None:
                            emit_pv_dve(*pe_part)
                        if stage >= 7 and len(ETab) == 2:
                            pending = (i, pr, ETab)
                if stage >= 7 and pending is not None:
                    pe_part = emit_pv_pe(*pending)
                    emit_pv_dve(*pe_part)

            # ================= output projection (tail) =================
            if stage >= 8:
                with tc.tile_pool(name="yp", bufs=yp_bufs, space="PSUM") as ypp:
                    for i in range(NQT):
                        for nhalf in range(2):
                            yps = ypp.tile([P, KT], f32, tag="yps")
                            for kc in range(2):
                                nc.tensor.matmul(
                                    yps[:],
                                    OTs[kc][:, i * P:(i + 1) * P],
                                    wo_sb[:, kc, nhalf * KT:(nhalf + 1) * KT],
                                    start=(kc == 0), stop=(kc == 1),
                                )
                            ysb_t = ysb.tile([P, KT], f16, tag="ysb")
                            nc.any.tensor_copy(ysb_t[:], yps[:])
                            nc.sync.dma_start(
                                y[i * P:(i + 1) * P,
                                  nhalf * KT:(nhalf + 1) * KT],
                                ysb_t[:],
                            )

            if loop_ctx is not None:
                loop_ctx.__exit__(None, None, None)

    nc.compile()
    return nc


_NC_CACHE = {}


def _get_nc(reps=1, **kw):
    key = (reps, tuple(sorted(kw.items())))
    if key not in _NC_CACHE:
        _NC_CACHE[key] = _build(reps, **kw)
    return _NC_CACHE[key]


def make_core_inputs(q, k, v, w_q, b_q, w_k, b_k, w_v, b_v, w_o):
    """Host-side shard prep: list of 8 per-core input dicts."""
    import ml_dtypes
    f16 = np.dtype(_FP16)
    if _FP16 == "bfloat16":
        f16 = np.dtype(ml_dtypes.bfloat16)
    tri = np.triu(np.full((P, P), NEG, np.float32), k=1)
    in_maps = []
    for c in range(8):
        b, hg = c // 4, c % 4
        sl = slice(hg * DHG, (hg + 1) * DHG)
        wq_s = (w_q[sl] * SCALE).astype(np.float32)
        bq_s = (b_q[sl] * SCALE).astype(np.float32)
        wk_s = w_k[sl].astype(np.float32)
        bk_s = b_k[sl].astype(np.float32)
        wv_s = w_v[sl].astype(np.float32)
        bv_s = b_v[sl].astype(np.float32)
        fqk = f16 if QK_F16 else np.float32
        in_maps.append({
            "qT": np.ascontiguousarray(q[b].T).astype(fqk),
            "kT": np.ascontiguousarray(k[b].T).astype(fqk),
            "vT": np.ascontiguousarray(v[b].T).astype(f16),
            "wqT": np.ascontiguousarray(wq_s.T).astype(fqk),
            "wkT": np.ascontiguousarray(wk_s.T).astype(fqk),
            "wvT": np.ascontiguousarray(wv_s.T).astype(f16),
            "woT": np.ascontiguousarray(w_o[:, sl].T).astype(f16),
            "bqc": np.ascontiguousarray(bq_s.reshape(2, P).T),
            "bkc": np.ascontiguousarray(bk_s.reshape(2, P).T),
            "bvr": bv_s.reshape(1, DHG).astype(f16),
            "mask": tri,
            })
    return in_maps


def kernel(k, q, v, mask, w_k, b_k, w_q, b_q, w_v, b_v, w_o, b_o):
    """Full-input entry point. mask is 1 (causal) per the reference."""
    from concourse.bass_utils import run_bass_kernel_spmd

    q = np.asarray(q, np.float32)
    k = np.asarray(k, np.float32)
    v = np.asarray(v, np.float32)
    w_q = np.asarray(w_q, np.float32); b_q = np.asarray(b_q, np.float32)
    w_k = np.asarray(w_k, np.float32); b_k = np.asarray(b_k, np.float32)
    w_v = np.asarray(w_v, np.float32); b_v = np.asarray(b_v, np.float32)
    w_o = np.asarray(w_o, np.float32); b_o = np.asarray(b_o, np.float32)

    nc = _get_nc(1)
    in_maps = make_core_inputs(q, k, v, w_q, b_q, w_k, b_k, w_v, b_v, w_o)
    res = run_bass_kernel_spmd(nc, in_maps, core_ids=list(range(8))).results
    out = np.zeros((B, S, D), np.float32)
    for c in range(8):
        out[c // 4] += res[c]["y"].astype(np.float32)
    out += b_o.astype(np.float32)
    return out

